# revision 1
# baseline (speedup 1.0000x reference)
"""Trainium2 Bass kernel for a 2-layer BCos-GCN (nn_BCosGCN_28346784153649).

Strategy (8 NeuronCores, SPMD), v2 — reassociated aggregation:

  GCN layer: out = dinv_dst * ((S @ T + T_own) @ W) + b,  T = dinv .* feat
  (S is the pure 0/1 edge one-hot; the dinv_src factor is folded into the
  table rows, the self-loop folds into the same PSUM-accumulated sum.)

  - Layer 1 table T1 = dinv.*x is a kernel INPUT, staged replicated in every
    core's DRAM in "residue bank" layout -> NO AllGather before layer 1; the
    per-edge gathers start immediately at kernel start.
  - Layer 2 table T2 = dinv.*h1 is produced by the layer-1 epilogue and
    AllGathered in two block-halves per residue bank (8 small collectives)
    that mostly overlap the tail of layer 1.
  - Aggregation per 128-dst-node block: dma_gather of source rows (int16
    indices into 4 fp16 residue-bank tables, 2 groups = 8 blocks per call),
    then 16 one-hot matmuls PSUM-accumulate U^T = (S@T)^T directly
    (stationary = gathered rows, moving = fp8 one-hot -> no U transpose);
    the self-loop row T_own^T is added during the PSUM->SBUF copy and the
    result feeds the replicated 128x128 weight matmul; LayerNorm + ELU
    (+ BCos mix / mean-pool one-hots in layer 2) run on vector/scalar.
  - The one-hot S chunks are built ON-CHIP (iota vs. label is_equal on the
    DVE) from a compact [128, 1600] fp16 label table - nothing but the
    gathered rows and tiny index/label tables ever stream from HBM.
  - Global mean-pool partial logits ([128,10] per core) are combined on the
    host (scatter-add by per-core graph base, divide by counts, add bias).

Host-side preprocessing (cached GCN normalization dinv, node placement,
edge bucketing/padding, index tables) is numpy; all heavy per-edge/per-node
compute runs on the NeuronCores.
"""

import sys

sys.path.insert(0, "/opt/trn_rl_repo")

import numpy as np

from concourse import bacc, tile, mybir
from concourse.bass_utils import run_bass_kernel_spmd
from concourse.masks import make_identity

# ---------------------------------------------------------------- constants
N, E, F, H, C, G = 100000, 1600000, 128, 128, 10, 512
LN_EPS = 1e-5
BCOS_EPS = 1e-6
TEMP = 1.5
RR = 0.6  # residual ratio; bcos exponent B == 1.0 -> bcos(h) = TEMP*h/(nrm+eps)

NCORES = 8
P = 128
REAL_PER_CORE = N // NCORES            # 12500
NODES_PER_CORE = 12800                 # padded: 100 blocks of 128
BLOCKS_PER_CORE = NODES_PER_CORE // P  # 100
NPAD = NODES_PER_CORE * NCORES         # 102400
NBLK = NPAD // P                       # 800
RES = 4                                # residue banks (slot>>5)
B_GRP = 4                              # dst blocks per group / PSUM tile
N_GRP = BLOCKS_PER_CORE // B_GRP       # 25 groups per core
ROWS_PER_BANK = NPAD // RES            # 25600 (< int16 max)

# table halves (for the layer-2 AllGather split): blocks [0,48) and [48,100)
HALF_BLK = [(0, 48), (48, 100)]
HALF_ROWS = [(b1 - b0) * 32 for b0, b1 in HALF_BLK]      # [1536, 1664]
HALF_BASE = [0, NCORES * HALF_ROWS[0]]                   # [0, 12288]
HALF_OF_GROUP = [0 if g < 12 else 1 for g in range(N_GRP)]

F16 = mybir.dt.float16
F32 = mybir.dt.float32
F8 = mybir.dt.float8e4
I16 = mybir.dt.int16
I32 = mybir.dt.int32
AOp = mybir.AluOpType
Act = mybir.ActivationFunctionType
AxX = mybir.AxisListType.X


# ---------------------------------------------------------------- host prep
def _lpt_blocks(indeg_core: np.ndarray) -> list[list[int]]:
    """Pack the core's real nodes into 100 blocks of <=128, balancing the
    in-degree sum per block (greedy LPT)."""
    import heapq

    order = np.argsort(-indeg_core, kind="stable")
    heap = [(0, 0, b) for b in range(BLOCKS_PER_CORE)]
    heapq.heapify(heap)
    blocks: list[list[int]] = [[] for _ in range(BLOCKS_PER_CORE)]
    for v in order:
        while True:
            load, cnt, b = heapq.heappop(heap)
            if cnt < P:
                break
        blocks[b].append(int(v))
        heapq.heappush(heap, (load + int(indeg_core[v]), cnt + 1, b))
    return blocks


def _color_banks(ownblk, src, dstblk, rounds=24, seed=0):
    """Greedy residue-bank coloring balancing (dst-block, color) edge cells
    at <=512 (-> K=4), subject to <=32 nodes per (own-block, color)."""
    SLOT_CAP = P // RES
    Nn = ownblk.shape[0]
    rng = np.random.default_rng(seed)
    eorder = np.argsort(src, kind="stable")
    e_dstblk = dstblk[eorder]
    esrc = src[eorder]
    degn = np.bincount(src, minlength=Nn)
    estart = np.concatenate([[0], np.cumsum(degn)])
    cellcnt = np.zeros((NBLK, RES), np.int64)
    slotcnt = np.zeros((NBLK, RES), np.int32)
    color = np.full(Nn, -1, np.int32)
    order = np.argsort(-degn, kind="stable")
    target = max(1.0, dstblk.shape[0] / (NBLK * RES))
    cap = int(np.ceil(target / P) * P)
    for bt in np.array_split(order, rounds):
        nb = bt.shape[0]
        reps = degn[bt]
        node_rep = np.repeat(np.arange(nb), reps)
        eidx = (np.concatenate([np.arange(estart[v], estart[v + 1]) for v in bt])
                if nb else np.empty(0, np.int64))
        score = np.zeros((nb, RES), np.float64)
        if eidx.size:
            np.add.at(score, node_rep, cellcnt[e_dstblk[eidx]])
        own = ownblk[bt]
        score += np.where(slotcnt[own] >= SLOT_CAP, 1e12, 0.0)
        if eidx.size:
            np.add.at(score, node_rep,
                      np.where(cellcnt[e_dstblk[eidx]] >= cap - 1, 1e6, 0.0))
        score += rng.random((nb, RES))
        ch = np.argmin(score, axis=1).astype(np.int32)
        for i in range(nb):
            o, c = own[i], ch[i]
            if slotcnt[o, c] >= SLOT_CAP:
                c = int(np.argmin(slotcnt[o] + np.where(
                    slotcnt[o] >= SLOT_CAP, 10**9, 0)))
                ch[i] = c
            slotcnt[o, c] += 1
        color[bt] = ch
        if eidx.size:
            np.add.at(cellcnt, (e_dstblk[eidx], ch[node_rep]), 1)
    # exact repair: move nodes out of over-cap cells
    border = np.argsort(e_dstblk, kind="stable")
    bcnt = np.bincount(e_dstblk, minlength=NBLK)
    bstart = np.concatenate([[0], np.cumsum(bcnt)])
    for _ in range(40):
        over = np.argwhere(cellcnt > cap)
        if over.size == 0:
            break
        for bb, cc in over:
            while cellcnt[bb, cc] > cap:
                cands = np.unique(esrc[border[bstart[bb]:bstart[bb + 1]]])
                cands = cands[color[cands] == cc]
                moved = False
                contrib = np.array([
                    np.count_nonzero(e_dstblk[estart[v]:estart[v + 1]] == bb)
                    for v in cands])
                for v in cands[np.argsort(contrib)]:
                    o = ownblk[v]
                    blks = e_dstblk[estart[v]:estart[v + 1]]
                    for c2 in np.argsort(cellcnt[bb]):
                        if c2 == cc or slotcnt[o, c2] >= SLOT_CAP:
                            continue
                        add = np.bincount(blks, minlength=NBLK)
                        touched = np.nonzero(add)[0]
                        if (cellcnt[touched, c2] + add[touched] <= cap).all():
                            cellcnt[touched, cc] -= add[touched]
                            cellcnt[touched, c2] += add[touched]
                            slotcnt[o, cc] -= 1
                            slotcnt[o, c2] += 1
                            color[v] = c2
                            moved = True
                            break
                    if moved:
                        break
                if not moved:
                    break
    return color


def _pairs():
    ps = [(2 * i, 2 * i + 1) for i in range(N_GRP // 2)]
    if N_GRP % 2:
        ps.append((N_GRP - 1,))
    return ps


def _prep(x, src, dst, batch, W1, b1, ln1_w, ln1_b, W2, b2, ln2_w, ln2_b,
          cls_v, cls_g, cls_b, seed=0):
    indeg = np.bincount(dst, minlength=N)
    deg = indeg.astype(np.float32) + 1.0
    dinv = (1.0 / np.sqrt(deg)).astype(np.float32)

    # ---- node -> (core, block); LPT balance in-degree per block
    ownblk = np.zeros(N, np.int64)
    core_blocks = []
    g_base = np.zeros(NCORES, np.int64)
    for c in range(NCORES):
        lo, hi = c * REAL_PER_CORE, (c + 1) * REAL_PER_CORE
        g_base[c] = int(batch[lo])
        span = int(batch[hi - 1]) - g_base[c]
        assert span < P, f"core {c} spans {span + 1} graphs > 128"
        blocks = _lpt_blocks(indeg[lo:hi])
        core_blocks.append(blocks)
        for b in range(BLOCKS_PER_CORE):
            for v_local in blocks[b]:
                ownblk[lo + v_local] = c * BLOCKS_PER_CORE + b

    # ---- residue-bank coloring (cells <= 512 -> K=4); slot assignment
    s64 = src.astype(np.int64)
    d64 = dst.astype(np.int64)
    color = _color_banks(ownblk, s64, ownblk[d64])
    pos = np.full(N, -1, np.int64)
    for c in range(NCORES):
        lo = c * REAL_PER_CORE
        for b in range(BLOCKS_PER_CORE):
            blk = core_blocks[c][b]
            base = c * NODES_PER_CORE + b * P
            # color r occupies contiguous slots [32r, 32r+31] so each
            # residue class is a contiguous partition range
            nxt = [0, 0, 0, 0]
            for v_local in blk:
                cc = int(color[lo + v_local])
                sl = 32 * cc + nxt[cc]
                nxt[cc] += 1
                pos[lo + v_local] = base + sl
    assert (pos >= 0).all()

    # global padded position -> (core, local block, slot)
    pcore = pos // NODES_PER_CORE
    pblk = (pos % NODES_PER_CORE) // P
    pslot = pos % P

    # residue-bank row number (half-major table layout, same for both layers)
    def bank_row(core_a, blk_a, slot_a):
        half = (blk_a >= HALF_BLK[1][0]).astype(np.int64)
        base = np.where(half == 0, HALF_BASE[0], HALF_BASE[1])
        rows_h = np.where(half == 0, HALF_ROWS[0], HALF_ROWS[1])
        b0 = np.where(half == 0, HALF_BLK[0][0], HALF_BLK[1][0])
        return base + core_a * rows_h + (blk_a - b0) * 32 + (slot_a & 31)

    node_row = bank_row(pcore, pblk, pslot)          # row within its bank
    node_res = pslot >> 5

    # ---- per-position node data
    node_at = np.full(NPAD, -1, np.int64)
    node_at[pos] = np.arange(N)

    x16 = (x * dinv[:, None]).astype(np.float16)     # T1 rows = dinv .* x

    # replicated residue-bank tables of T1
    xtab = np.zeros((RES, ROWS_PER_BANK, F), np.float16)
    xtab[node_res, node_row] = x16

    # per-core own-rows TRANSPOSED [feat, block, slot], per-(slot,block)
    # dinv / batch label
    xs2 = np.zeros((NCORES, P, BLOCKS_PER_CORE, P), np.float16)
    d1t = np.ones((NCORES, P, BLOCKS_PER_CORE), np.float32)
    lbt = np.zeros((NCORES, P, BLOCKS_PER_CORE), np.float16)
    for c in range(NCORES):
        sel = node_at[c * NODES_PER_CORE:(c + 1) * NODES_PER_CORE]
        ok = sel >= 0
        xs_flat = np.zeros((NODES_PER_CORE, F), np.float16)
        xs_flat[ok] = x16[sel[ok]]
        xs2[c] = xs_flat.reshape(BLOCKS_PER_CORE, P, F).transpose(2, 0, 1)
        d1 = np.ones(NODES_PER_CORE, np.float32)
        d1[ok] = dinv[sel[ok]]
        d1t[c] = d1.reshape(BLOCKS_PER_CORE, P).T
        lb = np.zeros(NODES_PER_CORE, np.float32)
        lb[ok] = (batch[sel[ok]] - g_base[c]).astype(np.float32)
        lbt[c] = lb.reshape(BLOCKS_PER_CORE, P).T.astype(np.float16)

    # ---- edges -> cells (dst block x src residue class), padded to K*128
    pe_src = pos[s64]
    pe_dst = pos[d64]
    blk = pe_dst >> 7                                 # global dst block
    res = (pe_src & 127) >> 5
    idx16 = bank_row(pe_src // NODES_PER_CORE,
                     (pe_src % NODES_PER_CORE) // P,
                     pe_src % P).astype(np.int16)
    ld = (pe_dst & 127).astype(np.float16)
    cell = blk * RES + res
    counts = np.bincount(cell, minlength=NBLK * RES)
    K = int(np.ceil(counts.max() / P))
    CELL = K * P

    order = np.argsort(cell, kind="stable")
    starts = np.cumsum(counts) - counts
    within = np.arange(E) - np.repeat(starts, counts)
    flat = cell[order] * CELL + within
    idxA = np.zeros(NBLK * RES * CELL, np.int16)      # pad -> row 0 (weight 0)
    ldA = np.full(NBLK * RES * CELL, -1.0, np.float16)  # pad label -1
    idxA[flat] = idx16[order]
    ldA[flat] = ld[order]
    idxA = idxA.reshape(NBLK, RES, CELL)
    ldA = ldA.reshape(NBLK, RES, K, P)

    NCHUNK = RES * K
    # compact labels: ldc[core][j, b*NCHUNK + rr*K + k]
    ldc = np.zeros((NCORES, P, BLOCKS_PER_CORE * NCHUNK), np.float16)
    for c in range(NCORES):
        sl = ldA[c * BLOCKS_PER_CORE:(c + 1) * BLOCKS_PER_CORE]  # [100,R,K,P]
        ldc[c] = sl.transpose(3, 0, 1, 2).reshape(P, -1)

    # gather indices, 16-way wrapped, flat (pair-major, residue-minor)
    pairs = _pairs()
    total_cols = sum(len(pr) * B_GRP * CELL // 16 for pr in pairs) * RES
    idxw = np.zeros((NCORES, P, total_cols), np.int16)
    for c in range(NCORES):
        off = 0
        for pr in pairs:
            b0 = c * BLOCKS_PER_CORE + pr[0] * B_GRP
            nb = len(pr) * B_GRP
            for rr in range(RES):
                lst = idxA[b0:b0 + nb, rr, :].reshape(-1)
                cols = lst.shape[0] // 16
                w = lst.reshape(-1, 16).T        # slot i -> [i%16, i//16]
                idxw[c, :, off:off + cols] = np.tile(w, (8, 1))
                off += cols
        assert off == total_cols

    # ---- classifier / epilogue host data
    WnT = (cls_g[:, None] * cls_v
           / np.linalg.norm(cls_v, axis=1, keepdims=True)).T.astype(np.float16)
    cnt = np.maximum(np.bincount(batch, minlength=G).astype(np.float32), 1.0)

    trivial = dict(
        b1=not np.any(b1), b2=not np.any(b2),
        ln1=bool(np.all(ln1_w == 1.0) and not np.any(ln1_b)),
        ln2=bool(np.all(ln2_w == 1.0) and not np.any(ln2_b)),
    )
    return dict(
        K=K, xtab=xtab, xs2=xs2, d1t=d1t, lbt=lbt, ldc=ldc, idxw=idxw,
        WnT=WnT, cnt=cnt, g_base=g_base, trivial=trivial,
        W1h=W1.astype(np.float16), W2h=W2.astype(np.float16),
        b1=b1.astype(np.float32), b2=b2.astype(np.float32),
        ln1_w=ln1_w.astype(np.float32), ln1_b=ln1_b.astype(np.float32),
        ln2_w=ln2_w.astype(np.float32), ln2_b=ln2_b.astype(np.float32),
        cls_b=cls_b.astype(np.float32),
    )


# ---------------------------------------------------------------- program
def _build(K: int, trivial: dict, max_phase: int = 99):
    CELL = K * P
    NCHUNK = RES * K
    pairs = _pairs()
    npairs = len(pairs)
    # flat idx column offsets per (pair, rr)
    idx_off = {}
    off = 0
    for pi, pr in enumerate(pairs):
        cols = len(pr) * B_GRP * CELL // 16
        for rr in range(RES):
            idx_off[(pi, rr)] = (off, cols)
            off += cols
    TOTAL_IDX_COLS = off

    nc = bacc.Bacc(None, target_bir_lowering=False, debug=False,
                   num_devices=NCORES, num_swdge_queues=4)

    xtab_p = [nc.declare_dram_parameter(f"xtab{r}", [ROWS_PER_BANK, F], F16,
                                        isOutput=False) for r in range(RES)]
    xs2_p = nc.declare_dram_parameter("xs2", [P, BLOCKS_PER_CORE, F], F16,
                                      isOutput=False)
    W1_p = nc.declare_dram_parameter("W1h", [F, H], F16, isOutput=False)
    W2_p = nc.declare_dram_parameter("W2h", [H, H], F16, isOutput=False)
    idxw_p = nc.declare_dram_parameter("idxw", [P, TOTAL_IDX_COLS], I16,
                                       isOutput=False)
    ldc_p = nc.declare_dram_parameter(
        "ldc", [P, BLOCKS_PER_CORE * NCHUNK], F16, isOutput=False)
    d1t_p = nc.declare_dram_parameter("d1t", [P, BLOCKS_PER_CORE], F32,
                                      isOutput=False)
    lbt_p = nc.declare_dram_parameter("lbt", [P, BLOCKS_PER_CORE], F16,
                                      isOutput=False)
    WnT_p = nc.declare_dram_parameter("WnT", [H, C], F16, isOutput=False)
    b1_p = nc.declare_dram_parameter("b1r", [1, H], F32, isOutput=False)
    b2_p = nc.declare_dram_parameter("b2r", [1, H], F32, isOutput=False)
    ln1w_p = nc.declare_dram_parameter("ln1wr", [1, H], F32, isOutput=False)
    ln1b_p = nc.declare_dram_parameter("ln1br", [1, H], F32, isOutput=False)
    ln2w_p = nc.declare_dram_parameter("ln2wr", [1, H], F32, isOutput=False)
    ln2b_p = nc.declare_dram_parameter("ln2br", [1, H], F32, isOutput=False)
    out_p = nc.declare_dram_parameter("out_part", [P, C], F32, isOutput=True)

    with tile.TileContext(nc, num_cores=NCORES) as tc:
        with (
            tc.tile_pool(name="consts", bufs=1) as consts,
            tc.tile_pool(name="resident", bufs=1) as resident,
            tc.tile_pool(name="work", bufs=3) as work,
            tc.tile_pool(name="gat", bufs=2) as gatp,
            tc.tile_pool(name="sbp", bufs=8) as sbp,
            tc.tile_pool(name="psum_u", bufs=2, space="PSUM") as psum_u,
            tc.tile_pool(name="psum_y", bufs=2, space="PSUM") as psum_y,
            tc.tile_pool(name="psum_tr", bufs=2, space="PSUM") as psum_tr,
            tc.tile_pool(name="psum_poolg", bufs=1, space="PSUM") as psum_poolg,
            tc.tile_pool(name="dram", bufs=1, space="DRAM") as dram,
        ):
            # DRAM: layer-2 AllGather inputs (2 halves x 4 residues) + tables
            agin = [[dram.tile([HALF_ROWS[h] , H], F16, tag=f"agin{r}_{h}",
                               name=f"agin{r}_{h}") for h in range(2)]
                    for r in range(RES)]
            tables = [dram.tile([ROWS_PER_BANK, H], F16, tag=f"tab{r}",
                                name=f"tab{r}")
                      for r in range(RES)]
            agin_v = [[agin[r][h][:].rearrange("(b q) d -> q b d", q=32)
                       for h in range(2)] for r in range(RES)]
            import os
            if not os.environ.get("BASS_NO_COPY_XTAB"):
                # stage the replicated T1 banks into internal DRAM first
                xtabi = [dram.tile([ROWS_PER_BANK, H], F16, tag=f"xtabi{r}",
                                   name=f"xtabi{r}") for r in range(RES)]
                HB = ROWS_PER_BANK // 2
                for r in range(RES):
                    for hh in range(2):
                        nc.sync.dma_start(
                            out=xtabi[r][hh * HB:(hh + 1) * HB]
                            .rearrange("(a b) d -> a (b d)", b=128),
                            in_=xtab_p[r][hh * HB:(hh + 1) * HB]
                            .rearrange("(a b) d -> a (b d)", b=128))
            else:
                xtabi = xtab_p  # gather straight from the input tables

            # ---------------- constants
            W1_t = consts.tile([F, H], F16)
            nc.sync.dma_start(out=W1_t[:], in_=W1_p[:])
            W2_t = consts.tile([H, H], F16)
            nc.sync.dma_start(out=W2_t[:], in_=W2_p[:])
            d1t_t = consts.tile([P, BLOCKS_PER_CORE], F32)
            nc.sync.dma_start(out=d1t_t[:], in_=d1t_p[:])
            lbt_t = consts.tile([P, BLOCKS_PER_CORE], F16)
            nc.sync.dma_start(out=lbt_t[:], in_=lbt_p[:])
            WnT_t = consts.tile([H, C], F16)
            nc.sync.dma_start(out=WnT_t[:], in_=WnT_p[:])
            # idx/label tables gate the first gathers: load them on the
            # scalar engine's queue so they overlap the T1 staging copies
            ldc_t = consts.tile([P, BLOCKS_PER_CORE * NCHUNK], F16)
            nc.scalar.dma_start(out=ldc_t[:], in_=ldc_p[:])
            idx_all = consts.tile([P, TOTAL_IDX_COLS], I16)
            nc.scalar.dma_start(out=idx_all[:], in_=idxw_p[:])
            rows = {}
            for nm, pp in [("b1", b1_p), ("b2", b2_p), ("ln1w", ln1w_p),
                           ("ln1b", ln1b_p), ("ln2w", ln2w_p), ("ln2b", ln2b_p)]:
                t = consts.tile([1, H], F32, tag=f"row_{nm}")
                nc.sync.dma_start(out=t[:], in_=pp[:])
                rows[nm] = t

            # iota consts: chunk labels (0..127 per chunk) and pool labels
            # (values <= 127 are exact in fp16)
            iota_c = consts.tile([P, NCHUNK * P], F16)
            nc.gpsimd.iota(iota_c[:], pattern=[[0, NCHUNK], [1, P]], base=0,
                           channel_multiplier=0,
                           allow_small_or_imprecise_dtypes=True)
            iota_p = consts.tile([P, B_GRP * P], F16)
            nc.vector.tensor_copy(out=iota_p[:], in_=iota_c[:, :B_GRP * P])

            ident_h = consts.tile([P, P], F16)
            make_identity(nc, ident_h[:])
            bcos_eps_t = consts.tile([P, 1], F32)
            nc.vector.memset(bcos_eps_t[:], BCOS_EPS)
            ln_eps_t = consts.tile([P, 1], F32)
            nc.vector.memset(ln_eps_t[:], LN_EPS)
            ones_c = consts.tile([P, 1], F32)
            nc.vector.memset(ones_c[:], 1.0)
            zeros_c = consts.tile([P, 1], F32)
            nc.vector.memset(zeros_c[:], 0.0)

            # layer-1 output table rows (dinv.*h), resident for layer 2
            hs_groups = [resident.tile([P, B_GRP * H], F16, tag=f"hsg{g}",
                                       name=f"hsg{g}")
                         for g in range(N_GRP)]

            # tiny warmup AllGather: absorbs the cold-start cost of the
            # collective path while the T1 staging copy runs
            if max_phase >= 2:
                wu_in = dram.tile([1, H], F16, tag="wu_in", name="wu_in")
                wu_out = dram.tile([NCORES, H], F16, tag="wu_out",
                                   name="wu_out")
                wu_s = consts.tile([1, H], F16, tag="wu_s")
                nc.vector.memset(wu_s[:], 0.0)
                nc.sync.dma_start(out=wu_in[:], in_=wu_s[:])
                nc.gpsimd.collective_compute(
                    "AllGather", AOp.bypass,
                    replica_groups=[list(range(NCORES))],
                    ins=[wu_in[:].opt()], outs=[wu_out[:].opt()])

            gt_tiles = {}

            def issue_gathers(lyr, pi):
                pr = pairs[pi]
                nrows = len(pr) * B_GRP * CELL
                nch = nrows // P
                for rr in range(RES):
                    o, cols = idx_off[(pi, rr)]
                    gt = gatp.tile([P, 2 * B_GRP * K, H], F16,
                                   tag=f"gat{rr}", name=f"gat{rr}", bufs=2)
                    src = xtabi[rr] if lyr == 1 else tables[rr]
                    nc.gpsimd.dma_gather(
                        out_ap=gt[:, :nch, :], in_ap=src[:],
                        idxs_ap=idx_all[:, o:o + cols],
                        num_idxs=nrows, num_idxs_reg=nrows,
                        elem_size=H, elem_step=H, single_packet=False,
                        queue_num=rr,
                    )
                    gt_tiles[(lyr, pi, rr)] = gt

            def build_S(g):
                """On-chip one-hot chunks for the 4 blocks of group g."""
                out = []
                for bl in range(B_GRP):
                    b = g * B_GRP + bl
                    sb = sbp.tile([P, NCHUNK * P], F8, tag="sb", name="sb",
                                  bufs=8)
                    nc.vector.tensor_tensor(
                        out=sb[:].rearrange("p (c m) -> p c m", m=P),
                        in0=iota_c[:].rearrange("p (c m) -> p c m", m=P),
                        in1=ldc_t[:, b * NCHUNK:(b + 1) * NCHUNK]
                        .to_broadcast([P, NCHUNK, P]),
                        op=AOp.is_equal)
                    out.append(sb)
                return out

            sb_map = {}
            u_ps = {}
            own_map = {}

            def agg_mm(lyr, g):
                """PSUM-accumulate U = S @ T for the 4 blocks of group g."""
                pi = g // 2
                if lyr == 1:
                    own = work.tile([P, B_GRP * F], F16, tag="own", bufs=3)
                    nc.sync.dma_start(
                        out=own[:].rearrange("p (b f) -> p b f", f=F),
                        in_=xs2_p[:, g * B_GRP:(g + 1) * B_GRP, :])
                    own_map[g] = own
                # U^T[f, m] += G_chunk^T @ S_chunk (stationary = gathered
                # rows, moving = fp8 one-hot -> half the operand fetch)
                ups = psum_u.tile([P, B_GRP * P], F32, space="PSUM", tag="u")
                u_ps[g] = ups
                bl2_0 = (g - pairs[pi][0]) * B_GRP
                for bl in range(B_GRP):
                    sb = sb_map[g][bl]
                    for rr in range(RES):
                        gt = gt_tiles[(lyr, pi, rr)]
                        for k in range(K):
                            j2 = rr * K + k
                            nc.tensor.matmul(
                                out=ups[:, bl * P:(bl + 1) * P],
                                lhsT=gt[:, (bl2_0 + bl) * K + k, :],
                                rhs=sb[:, j2 * P:(j2 + 1) * P],
                                start=(rr == 0 and k == 0),
                                stop=(rr == RES - 1 and k == K - 1),
                            )

            def finish(lyr, g, W_t, b_row, lnw_row, lnb_row, triv_b, triv_ln,
                       pool_ps):
                GH = B_GRP * H
                ups = u_ps.pop(g)
                if lyr == 1:
                    ownT = own_map.pop(g)
                else:
                    # own^T: transpose the resident hs blocks on the PE
                    ownT = work.tile([P, GH], F16, tag="own", bufs=3)
                    for bl in range(B_GRP):
                        trp = psum_tr.tile([P, P], F16, space="PSUM",
                                           tag="tr")
                        nc.tensor.transpose(
                            out=trp[:],
                            in_=hs_groups[g][:, bl * H:(bl + 1) * H],
                            identity=ident_h[:])
                        nc.scalar.activation(out=ownT[:, bl * P:(bl + 1) * P],
                                             in_=trp[:], func=Act.Copy)
                # U^T = (S@T)^T + T_own^T  (fp16, SBUF)
                u4 = work.tile([P, GH], F16, tag="u4")
                nc.vector.tensor_tensor(out=u4[:], in0=ups[:], in1=ownT[:],
                                        op=AOp.add)
                yps = psum_y.tile([P, GH], F32, space="PSUM", tag="y")
                for bl in range(B_GRP):
                    nc.tensor.matmul(out=yps[:, bl * H:(bl + 1) * H],
                                     lhsT=u4[:, bl * P:(bl + 1) * P],
                                     rhs=W_t[:], start=True, stop=True)

                # ---- epilogue: x dinv_dst (+bias), LayerNorm, ELU
                gsl = slice(g * B_GRP, (g + 1) * B_GRP)
                v4 = work.tile([P, GH], F16, tag="v4")
                nc.vector.tensor_tensor(
                    out=v4[:].rearrange("p (b d) -> p b d", d=H),
                    in0=yps[:].rearrange("p (b d) -> p b d", d=H),
                    in1=d1t_t[:, gsl].to_broadcast([P, B_GRP, H]),
                    op=AOp.mult)
                if not triv_b:
                    nc.vector.tensor_tensor(
                        out=v4[:], in0=v4[:],
                        in1=b_row[:].to_broadcast([P, GH]), op=AOp.add)
                ms4 = work.tile([P, B_GRP], F32, tag="ms4")
                nc.vector.tensor_reduce(
                    out=ms4[:], in_=v4[:].rearrange("p (b d) -> p b d", d=H),
                    axis=AxX, op=AOp.add)
                ng4 = work.tile([P, B_GRP], F32, tag="ng4")
                nc.scalar.activation(out=ng4[:], in_=ms4[:], func=Act.Copy,
                                     scale=-1.0 / H)
                c4 = work.tile([P, GH], F16, tag="c4")
                nc.vector.tensor_tensor(
                    out=c4[:].rearrange("p (b d) -> p b d", d=H),
                    in0=v4[:].rearrange("p (b d) -> p b d", d=H),
                    in1=ng4[:].to_broadcast([P, B_GRP, H]), op=AOp.add)
                # variance: square (scalar engine) + reduce (DVE)
                scr = work.tile([P, GH], F16, tag="scr")
                nc.scalar.activation(out=scr[:], in_=c4[:], func=Act.Square)
                vs4 = work.tile([P, B_GRP], F32, tag="vs4")
                nc.vector.tensor_reduce(
                    out=vs4[:], in_=scr[:].rearrange("p (b d) -> p b d", d=H),
                    axis=AxX, op=AOp.add)
                sd4 = work.tile([P, B_GRP], F32, tag="sd4")
                nc.scalar.activation(out=sd4[:], in_=vs4[:], func=Act.Sqrt,
                                     scale=1.0 / H, bias=ln_eps_t[:])
                r4 = work.tile([P, B_GRP], F32, tag="r4")
                nc.vector.reciprocal(out=r4[:], in_=sd4[:])
                hh4 = work.tile([P, GH], F16, tag="hh4")
                nc.vector.tensor_tensor(
                    out=hh4[:].rearrange("p (b d) -> p b d", d=H),
                    in0=c4[:].rearrange("p (b d) -> p b d", d=H),
                    in1=r4[:].to_broadcast([P, B_GRP, H]), op=AOp.mult)
                if not triv_ln:
                    for bl in range(B_GRP):
                        nc.vector.tensor_tensor(
                            out=hh4[:, bl * H:(bl + 1) * H],
                            in0=hh4[:, bl * H:(bl + 1) * H],
                            in1=lnw_row[:].to_broadcast([P, H]), op=AOp.mult)
                        nc.vector.tensor_tensor(
                            out=hh4[:, bl * H:(bl + 1) * H],
                            in0=hh4[:, bl * H:(bl + 1) * H],
                            in1=lnb_row[:].to_broadcast([P, H]), op=AOp.add)
                # ELU(h) = min(exp(h) - 1, relu(h))
                ex4 = work.tile([P, GH], F16, tag="ex4")
                nc.scalar.activation(out=ex4[:], in_=hh4[:], func=Act.Exp)
                em4 = work.tile([P, GH], F16, tag="em4")
                nc.vector.tensor_tensor(
                    out=em4[:], in0=ex4[:],
                    in1=ones_c[:].to_broadcast([P, GH]), op=AOp.subtract)
                rl4 = work.tile([P, GH], F16, tag="rl4")
                nc.vector.tensor_tensor(
                    out=rl4[:], in0=hh4[:],
                    in1=zeros_c[:].to_broadcast([P, GH]), op=AOp.max)
                if lyr == 1:
                    h4 = work.tile([P, GH], F16, tag="h4")
                    nc.vector.tensor_tensor(out=h4[:], in0=em4[:], in1=rl4[:],
                                            op=AOp.min)
                    # T2 rows = dinv .* h -> resident + AllGather input
                    hs4 = hs_groups[g]
                    nc.vector.tensor_tensor(
                        out=hs4[:].rearrange("p (b d) -> p b d", d=H),
                        in0=h4[:].rearrange("p (b d) -> p b d", d=H),
                        in1=d1t_t[:, gsl].to_broadcast([P, B_GRP, H]),
                        op=AOp.mult)
                    hf = HALF_OF_GROUP[g]
                    gb = g * B_GRP - HALF_BLK[hf][0]
                    for r in range(RES):
                        nc.sync.dma_start(
                            out=agin_v[r][hf][:, gb:gb + B_GRP, :],
                            in_=hs4[:].rearrange("p (b d) -> p b d", d=H)
                            [32 * r:32 * (r + 1)])
                else:
                    h4 = work.tile([P, GH], F16, tag="h4")
                    nc.vector.tensor_tensor(out=h4[:], in0=em4[:], in1=rl4[:],
                                            op=AOp.min)
                    # h_b = h * (RR + (1-RR)*TEMP / (||h|| + eps))
                    nc.scalar.activation(out=scr[:], in_=h4[:],
                                         func=Act.Square)
                    qs4 = work.tile([P, B_GRP], F32, tag="qs4")
                    nc.vector.tensor_reduce(
                        out=qs4[:],
                        in_=scr[:].rearrange("p (b d) -> p b d", d=H),
                        axis=AxX, op=AOp.add)
                    nrm4 = work.tile([P, B_GRP], F32, tag="nrm4")
                    nc.scalar.activation(out=nrm4[:], in_=qs4[:],
                                         func=Act.Sqrt, bias=bcos_eps_t[:])
                    nc.scalar.activation(out=nrm4[:], in_=nrm4[:],
                                         func=Act.Copy, bias=BCOS_EPS)
                    rcp4 = work.tile([P, B_GRP], F32, tag="rcp4")
                    nc.vector.reciprocal(out=rcp4[:], in_=nrm4[:])
                    fac4 = work.tile([P, B_GRP], F32, tag="fac4")
                    nc.scalar.activation(out=fac4[:], in_=rcp4[:],
                                         func=Act.Copy,
                                         scale=(1.0 - RR) * TEMP, bias=RR)
                    hb4 = work.tile([P, GH], F16, tag="hb4")
                    nc.vector.tensor_tensor(
                        out=hb4[:].rearrange("p (b d) -> p b d", d=H),
                        in0=h4[:].rearrange("p (b d) -> p b d", d=H),
                        in1=fac4[:].to_broadcast([P, B_GRP, H]), op=AOp.mult)
                    # mean-pool one-hot + PE accumulation into pool_ps
                    sp4 = work.tile([P, B_GRP * P], F16, tag="sp4", bufs=2)
                    nc.vector.tensor_tensor(
                        out=sp4[:].rearrange("p (b m) -> p b m", m=P),
                        in0=iota_p[:].rearrange("p (b m) -> p b m", m=P),
                        in1=lbt_t[:, gsl].to_broadcast([P, B_GRP, P]),
                        op=AOp.is_equal)
                    for bl in range(B_GRP):
                        b = g * B_GRP + bl
                        nc.tensor.matmul(
                            out=pool_ps[:], lhsT=sp4[:, bl * P:(bl + 1) * P],
                            rhs=hb4[:, bl * H:(bl + 1) * H],
                            start=(b == 0),
                            stop=(b == BLOCKS_PER_CORE - 1))

            def fire_ag(hf):
                for r in range(RES):
                    nc.gpsimd.collective_compute(
                        "AllGather", AOp.bypass,
                        replica_groups=[list(range(NCORES))],
                        ins=[agin[r][hf][:].opt()],
                        outs=[tables[r][HALF_BASE[hf]:
                                        HALF_BASE[hf]
                                        + NCORES * HALF_ROWS[hf]].opt()],
                    )

            def run_layer(lyr, W_t, b_row, lnw_row, lnb_row, triv_b, triv_ln,
                          pool_ps):
                issue_gathers(lyr, 0)
                issue_gathers(lyr, 1)
                sb_map[0] = build_S(0)
                for g in range(N_GRP):
                    if g % 2 == 0 and g // 2 + 2 < npairs:
                        issue_gathers(lyr, g // 2 + 2)
                    if g + 1 < N_GRP:
                        sb_map[g + 1] = build_S(g + 1)
                    agg_mm(lyr, g)
                    if lyr == 1 and g == 13 and max_phase >= 2:
                        fire_ag(0)
                    if g > 0:
                        finish(lyr, g - 1, W_t, b_row, lnw_row, lnb_row,
                               triv_b, triv_ln, pool_ps)
                    sb_map.pop(g - 1, None)
                finish(lyr, N_GRP - 1, W_t, b_row, lnw_row, lnb_row,
                       triv_b, triv_ln, pool_ps)
                if lyr == 1 and max_phase >= 2:
                    fire_ag(1)

            with nc.named_scope("layer1"):
                run_layer(1, W1_t, rows["b1"], rows["ln1w"], rows["ln1b"],
                          trivial["b1"], trivial["ln1"], None)

            if max_phase < 3:
                outt0 = work.tile([P, C], F32, tag="outt")
                nc.vector.memset(outt0[:], 0.0)
                nc.sync.dma_start(out=out_p[:], in_=outt0[:])
            else:
                pool_ps = psum_poolg.tile([P, H], F32, space="PSUM")
                with nc.named_scope("layer2"):
                    run_layer(2, W2_t, rows["b2"], rows["ln2w"],
                              rows["ln2b"], trivial["b2"], trivial["ln2"],
                              pool_ps)

                # -------- pooled partial -> transpose -> classifier
                with nc.named_scope("fin"):
                    pooled = work.tile([P, H], F16, tag="pooled")
                    nc.vector.tensor_copy(out=pooled[:], in_=pool_ps[:])
                    psT = psum_tr.tile([P, P], F16, space="PSUM", tag="tr")
                    nc.tensor.transpose(out=psT[:], in_=pooled[:],
                                        identity=ident_h[:])
                    pooledT = work.tile([P, P], F16, tag="pooledT")
                    nc.vector.tensor_copy(out=pooledT[:], in_=psT[:])
                    cls_ps = psum_y.tile([P, B_GRP * H], F32, space="PSUM",
                                         tag="y")
                    nc.tensor.matmul(out=cls_ps[:, :C], lhsT=pooledT[:],
                                     rhs=WnT_t[:], start=True, stop=True)
                    outt = work.tile([P, C], F32, tag="outt")
                    nc.vector.tensor_copy(out=outt[:], in_=cls_ps[:, :C])
                    nc.sync.dma_start(out=out_p[:], in_=outt[:])

    nc.finalize()
    return nc


_CACHE: dict = {}
LAST_RESULTS = None


def _ensure_ntff_hook():
    """Install the antenv.axon_hooks shim so trace=True captures NTFF
    profiles through the axon PJRT .so (the trimmed container lacks the
    module trn_boot expects)."""
    import sys as _sys
    import types

    if "antenv.axon_hooks" not in _sys.modules:
        mod = types.ModuleType("antenv.axon_hooks")
        holder = [None]
        mod.set_axon_ntff_profile_hook = lambda h: holder.__setitem__(0, h)
        mod.get_axon_ntff_profile_hook = lambda: holder[0]
        _sys.modules["antenv.axon_hooks"] = mod
        import antenv

        antenv.axon_hooks = mod
    from antenv.axon_hooks import (get_axon_ntff_profile_hook,
                                   set_axon_ntff_profile_hook)

    if get_axon_ntff_profile_hook() is None:
        from trn_agent_boot.trn_boot import _ntff_profile_via_ctypes

        h = _ntff_profile_via_ctypes("/opt/axon/libaxon_pjrt.so")
        if h is not None:
            set_axon_ntff_profile_hook(h)


def kernel(**inputs) -> np.ndarray:
    np_inputs = {k: np.asarray(v) for k, v in inputs.items()}
    prep = _prep(**np_inputs)
    K = prep["K"]
    import os
    max_phase = int(os.environ.get("BASS_MAX_PHASE", "99"))
    tkey = (K, max_phase, tuple(sorted(prep["trivial"].items())))
    if tkey not in _CACHE:
        _CACHE[tkey] = _build(K, prep["trivial"], max_phase)
    nc = _CACHE[tkey]

    in_maps = []
    for c in range(NCORES):
        m = dict(
            xs2=prep["xs2"][c], W1h=prep["W1h"], W2h=prep["W2h"],
            idxw=prep["idxw"][c], ldc=prep["ldc"][c], d1t=prep["d1t"][c],
            lbt=prep["lbt"][c], WnT=prep["WnT"],
            b1r=prep["b1"][None, :], b2r=prep["b2"][None, :],
            ln1wr=prep["ln1_w"][None, :], ln1br=prep["ln1_b"][None, :],
            ln2wr=prep["ln2_w"][None, :], ln2br=prep["ln2_b"][None, :],
        )
        for r in range(RES):
            m[f"xtab{r}"] = prep["xtab"][r]
        in_maps.append(m)
    import os
    trace = bool(os.environ.get("BASS_KERNEL_TRACE"))
    if trace:
        _ensure_ntff_hook()
    res = run_bass_kernel_spmd(nc, in_maps, core_ids=list(range(NCORES)),
                               trace=trace)
    global LAST_RESULTS
    LAST_RESULTS = res
    if trace and res.exec_time_ns is not None:
        print(f"HW exec time: {res.exec_time_ns} ns", flush=True)

    # host unshard: scatter-add partial logits by per-core graph base,
    # divide by graph node counts, add classifier bias
    out = np.zeros((G, C), np.float64)
    for c in range(NCORES):
        part = res.results[c]["out_part"].astype(np.float64)
        gb = int(prep["g_base"][c])
        hi = min(G, gb + P)
        out[gb:hi] += part[: hi - gb]
    out = out / prep["cnt"][:, None] + prep["cls_b"][None, :]
    return out.astype(np.float32)

